# revision 4
# baseline (speedup 1.0000x reference)
"""Single-head causal attention (B=8, T=2048, D=1024, H=128) on 8 TRN2 NeuronCores.

Sharding: one batch element per core (data-parallel over B).

Per-core algorithm, all matmuls in bf16 (full PE speed + FWL hides LDWEIGHTS;
fp32 PSUM accumulation keeps rel err ~1e-3):
  - host supplies x^T as [128, ND, T] bf16 and packed W as [128, ND, 3, H] bf16
  - Q^T, K^T = W^T @ x^T per 512-wide chunk (d-accumulated in PSUM, cast to
    bf16 in SBUF)
  - V computed directly in [k, h] layout: per 128-k-tile, x^T tile is the
    stationary operand, W_V streams (no PE transposes needed)
  - per chunk: S^T[k, q] = K^T_tile.T @ Q^T_chunk (PSUM f32), exp via ACT with
    scale (no max-subtraction: logits are O(+-6) for this distribution),
    causal mask on diagonal tiles via gpsimd affine_select (zero-fill),
    P^T tiles in bf16
  - PV transposed: O[q_tile, h] += P^T_tile.T @ V_tile per (k-tile, q-tile),
    PSUM-accumulated -> output is produced in [q, h] layout directly
  - denominators: pacc = sum_j P^T_j (DVE bf16 adds), per-q-tile N=1 matmul
    pacc_tile.T @ ones -> sums[q_part, 1], DVE reciprocal on [128, 4],
    tensor_scalar_mul broadcasts 1/sum along h -> O normalized, DMA out
  - emission is software-pipelined (PV lags scores by one k-tile) so PE,
    ACT, DVE and Pool overlap
"""
import numpy as np

B, T, D, H = 8, 2048, 1024, 128
ND = D // 128      # 8 d-tiles
NTK = T // 128     # 16 k-tiles
NCH = T // 512     # 4 q-chunks
SCALE = float(H) ** -0.5

_CACHE = {}


def _build():
    import concourse.bass as bass  # noqa: F401
    from concourse import bacc
    import concourse.mybir as mybir
    import concourse.tile as tile

    f32 = mybir.dt.float32
    bf16 = mybir.dt.bfloat16

    nc = bacc.Bacc("TRN2", target_bir_lowering=False)
    xt_d = nc.dram_tensor("xt", (128, ND, T), bf16, kind="ExternalInput")
    w_d = nc.dram_tensor("w", (128, ND, 3, H), bf16, kind="ExternalInput")
    o_d = nc.dram_tensor("o", (NCH, 128, 4, H), f32, kind="ExternalOutput")

    with tile.TileContext(nc) as tc:
        with (
            tc.tile_pool(name="sb", bufs=1) as sb,
            tc.tile_pool(name="ps", bufs=1, space="PSUM") as ps,
        ):
            # ---- loads ----
            # chunk 0 arrives as 8 per-d slices so the first projection can
            # start early; chunks 1-3 are one big striped DMA each.
            xt = sb.tile([128, ND, T], bf16, tag="xt")
            for d in range(ND):
                nc.sync.dma_start(xt[:, d, 0:512], xt_d[:, d, 0:512])
            w = sb.tile([128, ND, 3, H], bf16, tag="w")
            nc.scalar.dma_start(w[:], w_d[:])
            for ch in range(1, NCH):
                nc.scalar.dma_start(xt[:, :, ch * 512:(ch + 1) * 512],
                                    xt_d[:, :, ch * 512:(ch + 1) * 512])

            # ---- constants ----
            ones_f32 = sb.tile([128, 1], f32, tag="ones_f32")
            nc.gpsimd.memset(ones_f32[:], 1.0)
            ones_col = sb.tile([128, 1], bf16, tag="ones_col")
            nc.vector.tensor_copy(ones_col[:], ones_f32[:])
            # start=True clears a PSUM bank's has-written bits bank-wide, so
            # interleaved per-q-tile accumulation groups must NOT each open
            # with start=True; instead one zero matmul opens the whole bank.
            z512 = sb.tile([128, 512], bf16, tag="z512")
            nc.gpsimd.memset(z512[:], 0.0)

            # ---- persistent SBUF ----
            qt = sb.tile([128, T], bf16, tag="qt")        # Q^T [h, t]
            kt = sb.tile([128, T], bf16, tag="kt")        # K^T [h, t]
            v = sb.tile([128, NTK, H], bf16, tag="v")     # V [k, h] tiles

            def emit_chunk(ch):
                c0, c1 = ch * 512, (ch + 1) * 512
                # --- projections Q^T, K^T for this chunk ---
                for idx, dst in ((0, qt), (1, kt)):
                    acc = ps.tile([128, 512], f32, tag="proj", bufs=2,
                                  name=f"acc{ch}_{idx}")
                    for d in range(ND):
                        nc.tensor.matmul(acc[:], w[:, d, idx, :],
                                         xt[:, d, c0:c1],
                                         start=(d == 0), stop=(d == ND - 1))
                    with nc.allow_low_precision(reason="bf16 activations"):
                        nc.vector.tensor_copy(dst[:, c0:c1], acc[:])
                # --- V tiles directly in [k, h] layout ---
                vps = ps.tile([128, 512], f32, tag="proj", bufs=2,
                              name=f"vps{ch}")
                for i in range(4):
                    g = 4 * ch + i
                    for d in range(ND):
                        nc.tensor.matmul(vps[:, i * H:(i + 1) * H],
                                         xt[:, d, g * 128:(g + 1) * 128],
                                         w[:, d, 2, :],
                                         start=(d == 0), stop=(d == ND - 1))
                with nc.allow_low_precision(reason="bf16 activations"):
                    nc.vector.tensor_copy(v[:, 4 * ch:4 * ch + 4, :], vps[:])

                # --- attention ---
                o_ps = ps.tile([128, 512], f32, tag="ops", bufs=2,
                               name=f"ops{ch}")
                pacc = sb.tile([128, 512], bf16, tag="pacc", bufs=2,
                               name=f"pacc{ch}")
                nk = 4 * ch + 4

                pts = [None] * nk

                def emit_pv(j):
                    m = j - 4 * ch
                    for i in range(max(0, m), 4):
                        nc.tensor.matmul(
                            o_ps[:, i * H:(i + 1) * H],
                            pts[j][:, i * 128:(i + 1) * 128],
                            v[:, j, :],
                            start=False, stop=(j == 4 * ch + i),
                            skip_group_check=True,
                        )

                nc.tensor.matmul(o_ps[:], z512[:, 0:128], z512[:],
                                 start=True, stop=False, skip_group_check=True)

                for j in range(nk):
                    m = j - 4 * ch  # >= 0 on diagonal tiles
                    lo = 128 * m if m > 0 else 0
                    stp = ps.tile([128, 512], f32, tag="stp", bufs=3,
                                  name=f"stp{ch}_{j}")
                    nc.tensor.matmul(stp[:, lo:512], kt[:, j * 128:(j + 1) * 128],
                                     qt[:, c0 + lo:c1], start=True, stop=True)
                    pt = sb.tile([128, 512], bf16, tag="pt", bufs=6,
                                 name=f"pt{ch}_{j}")
                    pts[j] = pt
                    nc.scalar.activation(pt[:, lo:512], stp[:, lo:512],
                                         mybir.ActivationFunctionType.Exp,
                                         scale=SCALE)
                    if m >= 0:
                        if lo:
                            nc.gpsimd.memset(pt[:, 0:lo], 0.0)
                        nc.gpsimd.affine_select(
                            out=pt[:, lo:512], in_=pt[:, lo:512],
                            compare_op=mybir.AluOpType.is_ge, fill=0.0,
                            base=0, pattern=[[1, 512 - lo]],
                            channel_multiplier=-1,
                        )
                    if j > 0:
                        emit_pv(j - 1)  # PV lags so ACT overlaps PE
                    with nc.allow_low_precision(reason="bf16 denominator"):
                        if j == 0:
                            nc.vector.tensor_copy(pacc[:], pt[:])
                        else:
                            nc.vector.tensor_add(pacc[:], pacc[:], pt[:])
                emit_pv(nk - 1)

                # --- softmax denominators + normalize + store ---
                sums = ps.tile([128, 4], f32, tag="sums", name=f"sums{ch}")
                for i in range(4):
                    nc.tensor.matmul(sums[:, i:i + 1],
                                     pacc[:, i * 128:(i + 1) * 128],
                                     ones_col[:], start=True, stop=True)
                recip = sb.tile([128, 4], f32, tag="recip", bufs=2,
                                name=f"recip{ch}")
                nc.vector.reciprocal(recip[:], sums[:])
                osb = sb.tile([128, 4, H], f32, tag="osb", bufs=2,
                              name=f"osb{ch}")
                for i in range(4):
                    nc.vector.tensor_scalar_mul(osb[:, i, :],
                                                o_ps[:, i * H:(i + 1) * H],
                                                recip[:, i:i + 1])
                nc.sync.dma_start(o_d[ch], osb[:])

            for ch in range(NCH):
                emit_chunk(ch)

    nc.compile()
    return nc


def _in_maps(x, W_Q, W_V, W_K):
    import ml_dtypes

    bf16 = ml_dtypes.bfloat16
    # W packed [128, ND, 3, H]: w[p, d, s, h] = W_s[d*128+p, h]
    wall = np.stack([np.asarray(W_Q, np.float32),
                     np.asarray(W_K, np.float32),
                     np.asarray(W_V, np.float32)], 0)
    wall = np.ascontiguousarray(
        wall.reshape(3, ND, 128, H).transpose(2, 1, 0, 3)).astype(bf16)
    x = np.asarray(x, np.float32)
    maps = []
    for b in range(B):
        # x^T [128, ND, T]: xt[p, d, t] = x[b, t, d*128+p]
        xtr = np.ascontiguousarray(
            x[b].T.reshape(ND, 128, T).transpose(1, 0, 2)).astype(bf16)
        maps.append({"xt": xtr, "w": wall})
    return maps


def kernel(x, W_Q, W_K, W_V):
    from concourse import bass_utils

    if "nc" not in _CACHE:
        _CACHE["nc"] = _build()
    nc = _CACHE["nc"]

    in_maps = _in_maps(x, W_Q=W_Q, W_V=W_V, W_K=W_K)
    res = bass_utils.run_bass_kernel_spmd(nc, in_maps, core_ids=list(range(B)))
    out = np.stack([
        res.results[b]["o"].transpose(0, 2, 1, 3).reshape(T, H)
        for b in range(B)
    ]).astype(np.float32)
    return out
